# revision 27
# baseline (speedup 1.0000x reference)
"""Multi-head attention (B=2, S=2048, D=1024, H=16) on 8 Trainium2 cores.

Sharding: core = 4*b + g  (b = batch 0..1, g = head-group 0..3, 4 heads each).
Each core computes, for its batch b and head-group g (256 of the 1024 dims):
  QT/KT = (x @ W^T)^T  in [d, s] layout   (d on partitions)
  V     = x @ W^T      in [s, d] layout   (s on partitions)
  ST    = scores^T     in [k, q] layout   (k on partitions)  -> exp on ACT
  U     = V^T @ P^T    in [d, q] layout + per-head denominators Z via ones-matmul
  UN    = U / Z        (PE-broadcast reciprocal, DVE multiply)
  Ypart = UN^T @ WoT   in [q, e] layout   (partial over this group's 256 dims)
Host sums the 4 per-group partials per batch and adds b_o.

Schedule (v2): the ACT engine's exp stream (137us total) is the binding
resource, so everything is arranged to start it early and never starve it:
  - projections run Q -> K (shared PSUM ring), attention starts right after K
  - V is projected one sp-block per k-group, interleaved into the first
    attention pair's k-sweep on the PE queue
  - the two heads of a pair issue scores back-to-back at row groups 0/64
    (c=64 matmuls at disjoint tile_position rows execute concurrently)
  - U banks are drained to SBUF immediately at sweep end; the reciprocal/
    broadcast/multiply normalization runs off the critical path
  - output partials are written bf16 (host accumulates in f32)
"""

import os
from contextlib import ExitStack

import ml_dtypes
import numpy as np

import concourse.bass as bass
import concourse.tile as tile
from concourse import bacc, mybir
from concourse.tile import add_dep_helper

B, S, D = 2, 2048, 1024
H, DH = 16, 64
NCORES = 8
NG = 4                  # head-group shards
DG = D // NG            # 256 dims per head-group (4 heads)
P = 128
QC = 512                # q-chunk width
NQC = S // QC           # 4
NKT = S // P            # 16 k-tiles of 128
CD = D // P             # 8 contraction tiles for the projections
F32 = mybir.dt.float32
BF16 = mybir.dt.bfloat16
AF = mybir.ActivationFunctionType
SCALE = 1.0 / float(np.sqrt(D))


def _body(ctx: ExitStack, tc: "tile.TileContext", io: dict):
    nc = tc.nc
    ctx.enter_context(nc.allow_low_precision(reason="bf16 matmul pipeline"))
    sb = ctx.enter_context(tc.tile_pool(name="sb", bufs=1))

    QT, KT = {}, {}
    V = {}

    # --- phase 1: Q then K projections (shared 8-bank PSUM ring) -----------
    with tc.tile_pool(name="ps_proj", bufs=1, space="PSUM") as ps1:
        for nm, xkey, wkey, bkey, xtag, outmap in (
            ("q", "xq", "wq", "bq", "xq", QT),
            ("k", "xk", "wk", "bk", "xk", KT),
        ):
            w = sb.tile([P, CD, DG], BF16, tag=f"w{nm}", bufs=1, name=f"w{nm}")
            nc.sync.dma_start(w[:], io[wkey].rearrange("(c p) d -> p c d", p=P))
            psg = {}
            for d in range(2):
                for sc in range(NQC):
                    psg[d, sc] = ps1.tile(
                        [P, QC], F32, tag="proj", bufs=8, name=f"ps_{nm}{d}{sc}"
                    )
            bias = sb.tile([P, 2], F32, tag=f"b{nm}", bufs=1, name=f"b{nm}")
            xts = []
            for c in range(CD):
                xt = sb.tile([P, S], BF16, tag=xtag, bufs=8, name=f"x{nm}{c}")
                nc.sync.dma_start(xt[:], io[xkey][c * P : (c + 1) * P, :])
                xts.append(xt)
                if c == 0:
                    nc.sync.dma_start(
                        bias[:], io[bkey].rearrange("(t p) -> p t", p=P)
                    )
                for d in range(2):
                    for sc in range(NQC):
                        nc.tensor.matmul(
                            psg[d, sc][:],
                            (w[:, c, d * P : (d + 1) * P]),
                            (xt[:, sc * QC : (sc + 1) * QC]),
                            start=(c == 0),
                            stop=(c == CD - 1),
                        )
            if nm == "k":
                # issue V-phase + attention-phase DMAs while K computes
                wv = sb.tile([P, CD, DG], BF16, tag="wv", bufs=1, name="wv")
                nc.sync.dma_start(wv[:], io["wv"].rearrange("(c p) d -> p c d", p=P))
                xvs = []
                for c in range(CD):
                    xvt = sb.tile([P, S], BF16, tag="xv", bufs=8, name=f"xv{c}")
                    nc.sync.dma_start(xvt[:], io["xv"][c * P : (c + 1) * P, :])
                    xvs.append(xvt)
                ones_col = sb.tile([1, P], BF16, tag="ones_col", bufs=1, name="ones_col")
                nc.sync.dma_start(ones_col[:], io["ones"][None, :])
                ones4 = sb.tile([P, 4], BF16, tag="ones4", bufs=1, name="ones4")
                nc.sync.dma_start(ones4[:], io["ones4"][:])
                bv_row = sb.tile([1, DG], BF16, tag="bv", bufs=1, name="bv_row")
                nc.sync.dma_start(bv_row[:], io["bv"][None, :])
                woT = []
                for pr in range(2):
                    t = sb.tile([P, D], BF16, tag="wo", bufs=2, name=f"woT{pr}")
                    nc.sync.dma_start(t[:], io["wo"][pr * P : (pr + 1) * P, :])
                    woT.append(t)
            # d-outer bias order: pair 0's scores only need the d=0 tiles
            for d in range(2):
                for sc in range(NQC):
                    t = sb.tile([P, QC], BF16, tag=f"{nm}t", bufs=8, name=f"{nm}T{d}{sc}")
                    nc.vector.tensor_scalar_add(t[:], psg[d, sc][:], bias[:, d : d + 1])
                    outmap[d, sc] = t

    # --- phase 2: attention; V projection interleaved into (qc0, pair0) ----
    # PSUM budget: bank1(V-proj psv / out-proj yps) 2 + st 2x2 + u 2 = 8.
    ps2 = ctx.enter_context(tc.tile_pool(name="ps_attn", bufs=1, space="PSUM"))
    UN = {}
    YSB = {}
    Y0 = {}
    pending = []

    def v_block(sp):
        # V [2048, 256]: two 128-row s-tiles per PSUM bank; bias folded via
        # ones-matmul seeds. V_aug tiles [128, 4, 65]: per head 64 V columns
        # + a ones column that accumulates the softmax denominator.
        psv = ps2.tile([P, 2, DG], F32, tag="bank1", bufs=2, name=f"psv{sp}")
        seed = None
        for j in range(2):
            mm = nc.tensor.matmul(
                psv[:, j, :],
                (ones_col[:, 0:P]),
                (bv_row[:]),
                start=(j == 0),
                stop=False,
            )
            if j == 0:
                seed = mm
            else:
                add_dep_helper(mm.ins, seed.ins, reason="psum group order")
        last_j0 = None
        for c in range(CD):
            for j in range(2):
                st_i = sp * 2 + j
                mm = nc.tensor.matmul(
                    psv[:, j, :],
                    (xvs[c][:, st_i * P : (st_i + 1) * P]),
                    (wv[:, c, :]),
                    start=False,
                    stop=(c == CD - 1 and j == 1),
                )
                if j == 0:
                    last_j0 = mm
                elif c == CD - 1:
                    add_dep_helper(mm.ins, last_j0.ins, reason="psv stop order")
        for j in range(2):
            vt = sb.tile([P, 4, DH + 1], BF16, tag="v", bufs=16, name=f"V{sp}_{j}")
            nc.vector.tensor_copy(
                vt[:, :, 0:DH],
                psv[:, j, :].rearrange("p (g d) -> p g d", g=4),
            )
            nc.vector.tensor_copy(vt[:, :, DH : DH + 1], ones4[:, :, None])
            V[sp * 2 + j] = vt

    def emit_outproj_unit():
        if not pending:
            return
        qcp, qi, ec = pending.pop(0)
        qt = qcp * 4 + qi
        if ec == 0:
            YSB[qt] = sb.tile([P, D], BF16, tag="y", bufs=4, name=f"Y{qt}")
        ysb = YSB[qt]
        yps = ps2.tile([P, QC], F32, tag="bank1", bufs=2, name=f"yp{qt}_{ec}")
        for pr in range(2):
            nc.tensor.matmul(
                yps[:],
                (UN[qcp, pr][:, qi * P : (qi + 1) * P]),
                (woT[pr][:, ec * QC : (ec + 1) * QC]),
                start=(pr == 0),
                stop=(pr == 1),
            )
        nc.vector.tensor_copy(ysb[:, ec * QC : (ec + 1) * QC], yps[:])
        if ec == 1:
            nc.sync.dma_start(io["y"][qt * P : (qt + 1) * P, :], ysb[:])

    for qc in range(NQC):
        for pair in range(2):
            heads = (2 * pair, 2 * pair + 1)
            U = {
                h: ps2.tile([P, QC], F32, tag="u", bufs=2, name=f"U{qc}_{h}")
                for h in heads
            }
            def emit_pv(kg, pts):
                for h in heads:
                    for kk in range(2):
                        k_tile = kg * 2 + kk
                        nc.tensor.matmul(
                            U[h][0:65, :],
                            (V[k_tile][:, h, :]),
                            (pts[h][:, kk, :]),
                            start=(kg == 0 and kk == 0),
                            stop=(kg == 7 and kk == 1),
                        )

            pts_all = {}
            for kg in range(8):
                st = {
                    h: ps2.tile([P, 2, QC], F32, tag="st", bufs=2, name=f"st{qc}_{kg}_{h}")
                    for h in heads
                }
                # heads alternate row groups 0-63 / 64-127 -> the PE runs the
                # two c=64 scores matmuls of a kk concurrently
                s_last = None
                for kk in range(2):
                    k_tile = kg * 2 + kk
                    sc, off = divmod(k_tile, 4)
                    for h in heads:
                        pr, lo = h // 2, (h % 2) * 64
                        s_last = nc.tensor.matmul(
                            st[h][:, kk, :],
                            (KT[pr, sc][lo : lo + 64, off * P : (off + 1) * P]),
                            (QT[pr, qc][lo : lo + 64, :]),
                            start=True,
                            stop=True,
                            tile_position=(lo, 0),
                        )
                pts = {}
                for h in heads:
                    pt = sb.tile([P, 2, QC], BF16, tag="pt", bufs=8, name=f"pt{qc}_{kg}_{h}")
                    nc.scalar.activation(pt[:], st[h][:], AF.Exp, scale=SCALE)
                    pts[h] = pt
                pts_all[kg] = pts
                if qc == 0 and pair == 0:
                    v_block(kg)
                emit_pv(kg, pts)
                if kg % 2 == pair:
                    emit_outproj_unit()

            # drain U banks to SBUF immediately (frees PSUM for the next
            # pair); normalize off the critical path
            ucp = {}
            for h in heads:
                u = sb.tile([65, QC], F32, tag="ucp", bufs=4, name=f"ucp{qc}_{h}")
                nc.vector.tensor_copy(u[:], U[h][0:65, :])
                ucp[h] = u
            z2 = sb.tile([2, QC], F32, tag="z2", bufs=3, name=f"z2_{qc}_{pair}")
            for i, h in enumerate(heads):
                nc.sync.dma_start(z2[i : i + 1, :], ucp[h][64:65, :])
            rz2 = sb.tile([2, QC], F32, tag="rz2", bufs=3, name=f"rz2_{qc}_{pair}")
            nc.vector.reciprocal_approx_fast(rz2[:], z2[:])
            UN[qc, pair] = sb.tile([P, QC], BF16, tag="un", bufs=8, name=f"UN{qc}_{pair}")
            for i, h in enumerate(heads):
                off = (h % 2) * 64
                if i == 0:
                    r0 = rz2[0:1, :]
                else:
                    r0t = sb.tile([1, QC], F32, tag="r0", bufs=3, name=f"r0_{qc}_{h}")
                    nc.sync.dma_start(r0t[:], rz2[1:2, :])
                    r0 = r0t[:]
                rb = sb.tile([64, QC], F32, tag="rb", bufs=4, name=f"rb{qc}_{h}")
                nc.gpsimd.partition_broadcast(rb[:], r0, channels=64)
                if off == 0:
                    nc.vector.tensor_mul(UN[qc, pair][0:64, :], ucp[h][0:64, :], rb[:])
                else:
                    tmp = sb.tile([64, QC], BF16, tag="untmp", bufs=3, name=f"untmp{qc}_{h}")
                    nc.vector.tensor_mul(tmp[:], ucp[h][0:64, :], rb[:])
                    nc.sync.dma_start(UN[qc, pair][64:128, :], tmp[:])

        pending.extend((qc, qi, ec) for qi in range(4) for ec in range(2))

    while pending:
        emit_outproj_unit()


def build_program():
    nc = bacc.Bacc(
        "TRN2", target_bir_lowering=False, debug=False, num_devices=NCORES
    )
    io = {
        "xq": nc.dram_tensor("xq", [D, S], BF16, kind="ExternalInput").ap(),
        "xk": nc.dram_tensor("xk", [D, S], BF16, kind="ExternalInput").ap(),
        "xv": nc.dram_tensor("xv", [D, S], BF16, kind="ExternalInput").ap(),
        "wq": nc.dram_tensor("wq", [D, DG], BF16, kind="ExternalInput").ap(),
        "wk": nc.dram_tensor("wk", [D, DG], BF16, kind="ExternalInput").ap(),
        "wv": nc.dram_tensor("wv", [D, DG], BF16, kind="ExternalInput").ap(),
        "wo": nc.dram_tensor("wo", [DG, D], BF16, kind="ExternalInput").ap(),
        "bq": nc.dram_tensor("bq", [DG], F32, kind="ExternalInput").ap(),
        "bk": nc.dram_tensor("bk", [DG], F32, kind="ExternalInput").ap(),
        "bv": nc.dram_tensor("bv", [DG], BF16, kind="ExternalInput").ap(),
        "ones": nc.dram_tensor("ones", [P], BF16, kind="ExternalInput").ap(),
        "ones4": nc.dram_tensor("ones4", [P, 4], BF16, kind="ExternalInput").ap(),
        "y": nc.dram_tensor("y", [S, D], BF16, kind="ExternalOutput").ap(),
    }
    with tile.TileContext(nc) as tc:
        with ExitStack() as ctx:
            _body(ctx, tc, io)
    nc.compile()
    return nc


_CACHE = {}


def _get_program():
    if "nc" not in _CACHE:
        _CACHE["nc"] = build_program()
    return _CACHE["nc"]


def make_in_maps(inputs):
    q = np.asarray(inputs["query"], np.float32)
    k = np.asarray(inputs["key"], np.float32)
    v = np.asarray(inputs["value"], np.float32)
    W_q = np.asarray(inputs["W_q"], np.float32)
    W_k = np.asarray(inputs["W_k"], np.float32)
    W_v = np.asarray(inputs["W_v"], np.float32)
    W_o = np.asarray(inputs["W_o"], np.float32)
    b_q = np.asarray(inputs["b_q"], np.float32)
    b_k = np.asarray(inputs["b_k"], np.float32)
    b_v = np.asarray(inputs["b_v"], np.float32)

    bf = ml_dtypes.bfloat16
    xT = [
        [np.ascontiguousarray(x[b].T).astype(bf) for b in range(B)]
        for x in (q, k, v)
    ]
    in_maps = []
    for core in range(NCORES):
        b, g = divmod(core, NG)
        sl = slice(g * DG, (g + 1) * DG)
        in_maps.append(
            {
                "xq": xT[0][b],
                "xk": xT[1][b],
                "xv": xT[2][b],
                "wq": np.ascontiguousarray(W_q[sl, :].T).astype(bf),
                "wk": np.ascontiguousarray(W_k[sl, :].T).astype(bf),
                "wv": np.ascontiguousarray(W_v[sl, :].T).astype(bf),
                "wo": np.ascontiguousarray(W_o[:, sl].T).astype(bf),
                "bq": np.ascontiguousarray(b_q[sl]),
                "bk": np.ascontiguousarray(b_k[sl]),
                "bv": np.ascontiguousarray(b_v[sl]).astype(bf),
                "ones": np.ones(P, bf),
                "ones4": np.ones((P, 4), bf),
            }
        )
    return in_maps


def kernel(**inputs):
    from concourse.bass_utils import run_bass_kernel_spmd

    nc = _get_program()
    in_maps = make_in_maps(inputs)
    trace = bool(int(os.environ.get("MHA_TRACE", "0")))
    res = run_bass_kernel_spmd(nc, in_maps, list(range(NCORES)), trace=trace)
    _CACHE["last_results"] = res

    b_o = np.asarray(inputs["b_o"], np.float32)
    out = np.zeros((B, S, D), np.float32)
    for core in range(NCORES):
        b = core // NG
        out[b] += res.results[core]["y"].astype(np.float32)
    out += b_o[None, None, :]
    return out


# revision 28
# speedup vs baseline: 1.0003x; 1.0003x over previous
"""Multi-head attention (B=2, S=2048, D=1024, H=16) on 8 Trainium2 cores.

Sharding: core = 4*b + g  (b = batch 0..1, g = head-group 0..3, 4 heads each).
Each core computes, for its batch b and head-group g (256 of the 1024 dims):
  QT/KT = (x @ W^T)^T  in [d, s] layout   (d on partitions)
  V     = x @ W^T      in [s, d] layout   (s on partitions)
  ST    = scores^T     in [k, q] layout   (k on partitions)  -> exp on ACT
  U     = V^T @ P^T    in [d, q] layout + per-head denominators Z via ones-matmul
  UN    = U / Z        (PE-broadcast reciprocal, DVE multiply)
  Ypart = UN^T @ WoT   in [q, e] layout   (partial over this group's 256 dims)
Host sums the 4 per-group partials per batch and adds b_o.

Schedule (v2): the ACT engine's exp stream (137us total) is the binding
resource, so everything is arranged to start it early and never starve it:
  - projections run Q -> K (shared PSUM ring), attention starts right after K
  - V is projected one sp-block per k-group, interleaved into the first
    attention pair's k-sweep on the PE queue
  - the two heads of a pair issue scores back-to-back at row groups 0/64
    (c=64 matmuls at disjoint tile_position rows execute concurrently)
  - U banks are drained to SBUF immediately at sweep end; the reciprocal/
    broadcast/multiply normalization runs off the critical path
  - output partials are written bf16 (host accumulates in f32)
"""

import os
from contextlib import ExitStack

import ml_dtypes
import numpy as np

import concourse.bass as bass
import concourse.tile as tile
from concourse import bacc, mybir
from concourse.tile import add_dep_helper

B, S, D = 2, 2048, 1024
H, DH = 16, 64
NCORES = 8
NG = 4                  # head-group shards
DG = D // NG            # 256 dims per head-group (4 heads)
P = 128
QC = 512                # q-chunk width
NQC = S // QC           # 4
NKT = S // P            # 16 k-tiles of 128
CD = D // P             # 8 contraction tiles for the projections
F32 = mybir.dt.float32
BF16 = mybir.dt.bfloat16
AF = mybir.ActivationFunctionType
SCALE = 1.0 / float(np.sqrt(D))


def _body(ctx: ExitStack, tc: "tile.TileContext", io: dict):
    nc = tc.nc
    ctx.enter_context(nc.allow_low_precision(reason="bf16 matmul pipeline"))
    sb = ctx.enter_context(tc.tile_pool(name="sb", bufs=1))

    QT, KT = {}, {}
    V = {}

    # --- phase 1: Q then K projections (shared 8-bank PSUM ring) -----------
    with tc.tile_pool(name="ps_proj", bufs=1, space="PSUM") as ps1:
        for nm, xkey, wkey, bkey, xtag, outmap in (
            ("q", "xq", "wq", "bq", "xq", QT),
            ("k", "xk", "wk", "bk", "xk", KT),
        ):
            w = sb.tile([P, CD, DG], BF16, tag=f"w{nm}", bufs=1, name=f"w{nm}")
            nc.sync.dma_start(w[:], io[wkey].rearrange("(c p) d -> p c d", p=P))
            psg = {}
            for d in range(2):
                for sc in range(NQC):
                    psg[d, sc] = ps1.tile(
                        [P, QC], F32, tag="proj", bufs=8, name=f"ps_{nm}{d}{sc}"
                    )
            bias = sb.tile([P, 2], F32, tag=f"b{nm}", bufs=1, name=f"b{nm}")
            xts = []
            for c in range(CD):
                xt = sb.tile([P, S], BF16, tag=xtag, bufs=8, name=f"x{nm}{c}")
                nc.sync.dma_start(xt[:], io[xkey][c * P : (c + 1) * P, :])
                xts.append(xt)
                if c == 0:
                    nc.sync.dma_start(
                        bias[:], io[bkey].rearrange("(t p) -> p t", p=P)
                    )
                for d in range(2):
                    for sc in range(NQC):
                        nc.tensor.matmul(
                            psg[d, sc][:],
                            (w[:, c, d * P : (d + 1) * P]),
                            (xt[:, sc * QC : (sc + 1) * QC]),
                            start=(c == 0),
                            stop=(c == CD - 1),
                        )
            if nm == "k":
                # issue V-phase + attention-phase DMAs while K computes
                wv = sb.tile([P, CD, DG], BF16, tag="wv", bufs=1, name="wv")
                nc.sync.dma_start(wv[:], io["wv"].rearrange("(c p) d -> p c d", p=P))
                xvs = []
                for c in range(CD):
                    xvt = sb.tile([P, S], BF16, tag="xv", bufs=8, name=f"xv{c}")
                    nc.sync.dma_start(xvt[:], io["xv"][c * P : (c + 1) * P, :])
                    xvs.append(xvt)
                ones_col = sb.tile([1, P], BF16, tag="ones_col", bufs=1, name="ones_col")
                nc.sync.dma_start(ones_col[:], io["ones"][None, :])
                ones4 = sb.tile([P, 4], BF16, tag="ones4", bufs=1, name="ones4")
                nc.sync.dma_start(ones4[:], io["ones4"][:])
                bv_row = sb.tile([1, DG], BF16, tag="bv", bufs=1, name="bv_row")
                nc.sync.dma_start(bv_row[:], io["bv"][None, :])
                woT = []
                for pr in range(2):
                    t = sb.tile([P, D], BF16, tag="wo", bufs=2, name=f"woT{pr}")
                    nc.sync.dma_start(t[:], io["wo"][pr * P : (pr + 1) * P, :])
                    woT.append(t)
            # d-outer bias order: pair 0's scores only need the d=0 tiles
            for d in range(2):
                for sc in range(NQC):
                    t = sb.tile([P, QC], BF16, tag=f"{nm}t", bufs=8, name=f"{nm}T{d}{sc}")
                    nc.vector.tensor_scalar_add(t[:], psg[d, sc][:], bias[:, d : d + 1])
                    outmap[d, sc] = t

    # --- phase 2: attention; V projection interleaved into (qc0, pair0) ----
    # PSUM budget: bank1(V-proj psv / out-proj yps) 2 + st 2x2 + u 2 = 8.
    ps2 = ctx.enter_context(tc.tile_pool(name="ps_attn", bufs=1, space="PSUM"))
    UN = {}
    YSB = {}
    Y0 = {}
    pending = []

    def v_block(sp, after_mm=None):
        # V [2048, 256]: two 128-row s-tiles per PSUM bank; bias folded via
        # ones-matmul seeds. V_aug tiles [128, 4, 65]: per head 64 V columns
        # + a ones column that accumulates the softmax denominator.
        psv = ps2.tile([P, 2, DG], F32, tag="bank1", bufs=2, name=f"psv{sp}")
        seed = None
        for j in range(2):
            mm = nc.tensor.matmul(
                psv[:, j, :],
                (ones_col[:, 0:P]),
                (bv_row[:]),
                start=(j == 0),
                stop=False,
            )
            if j == 0:
                seed = mm
                if after_mm is not None:
                    add_dep_helper(mm.ins, after_mm.ins, reason="v after scores")
            else:
                add_dep_helper(mm.ins, seed.ins, reason="psum group order")
        last_j0 = None
        for c in range(CD):
            for j in range(2):
                st_i = sp * 2 + j
                mm = nc.tensor.matmul(
                    psv[:, j, :],
                    (xvs[c][:, st_i * P : (st_i + 1) * P]),
                    (wv[:, c, :]),
                    start=False,
                    stop=(c == CD - 1 and j == 1),
                )
                if j == 0:
                    last_j0 = mm
                elif c == CD - 1:
                    add_dep_helper(mm.ins, last_j0.ins, reason="psv stop order")
        for j in range(2):
            vt = sb.tile([P, 4, DH + 1], BF16, tag="v", bufs=16, name=f"V{sp}_{j}")
            nc.vector.tensor_copy(
                vt[:, :, 0:DH],
                psv[:, j, :].rearrange("p (g d) -> p g d", g=4),
            )
            nc.vector.tensor_copy(vt[:, :, DH : DH + 1], ones4[:, :, None])
            V[sp * 2 + j] = vt

    def emit_outproj_unit():
        if not pending:
            return
        qcp, qi, ec = pending.pop(0)
        qt = qcp * 4 + qi
        if ec == 0:
            YSB[qt] = sb.tile([P, D], BF16, tag="y", bufs=4, name=f"Y{qt}")
        ysb = YSB[qt]
        yps = ps2.tile([P, QC], F32, tag="bank1", bufs=2, name=f"yp{qt}_{ec}")
        for pr in range(2):
            nc.tensor.matmul(
                yps[:],
                (UN[qcp, pr][:, qi * P : (qi + 1) * P]),
                (woT[pr][:, ec * QC : (ec + 1) * QC]),
                start=(pr == 0),
                stop=(pr == 1),
            )
        nc.vector.tensor_copy(ysb[:, ec * QC : (ec + 1) * QC], yps[:])
        if ec == 1:
            nc.sync.dma_start(io["y"][qt * P : (qt + 1) * P, :], ysb[:])

    for qc in range(NQC):
        for pair in range(2):
            heads = (2 * pair, 2 * pair + 1)
            U = {
                h: ps2.tile([P, QC], F32, tag="u", bufs=2, name=f"U{qc}_{h}")
                for h in heads
            }
            def emit_pv(kg, pts):
                for h in heads:
                    for kk in range(2):
                        k_tile = kg * 2 + kk
                        nc.tensor.matmul(
                            U[h][0:65, :],
                            (V[k_tile][:, h, :]),
                            (pts[h][:, kk, :]),
                            start=(kg == 0 and kk == 0),
                            stop=(kg == 7 and kk == 1),
                        )

            pts_all = {}
            for kg in range(8):
                st = {
                    h: ps2.tile([P, 2, QC], F32, tag="st", bufs=2, name=f"st{qc}_{kg}_{h}")
                    for h in heads
                }
                # heads alternate row groups 0-63 / 64-127 -> the PE runs the
                # two c=64 scores matmuls of a kk concurrently
                s_last = None
                for kk in range(2):
                    k_tile = kg * 2 + kk
                    sc, off = divmod(k_tile, 4)
                    for h in heads:
                        pr, lo = h // 2, (h % 2) * 64
                        s_last = nc.tensor.matmul(
                            st[h][:, kk, :],
                            (KT[pr, sc][lo : lo + 64, off * P : (off + 1) * P]),
                            (QT[pr, qc][lo : lo + 64, :]),
                            start=True,
                            stop=True,
                            tile_position=(lo, 0),
                        )
                pts = {}
                for h in heads:
                    pt = sb.tile([P, 2, QC], BF16, tag="pt", bufs=8, name=f"pt{qc}_{kg}_{h}")
                    nc.scalar.activation(pt[:], st[h][:], AF.Exp, scale=SCALE)
                    pts[h] = pt
                pts_all[kg] = pts
                if qc == 0 and pair == 0:
                    # V projection skewed one unit behind the scores: unit
                    # kg's scores are in flight before v_block(kg-1) may run
                    if kg >= 1:
                        v_block(kg - 1, after_mm=s_last)
                        emit_pv(kg - 1, pts_all[kg - 1])
                    if kg == 7:
                        v_block(7)
                        emit_pv(7, pts_all[7])
                    continue
                emit_pv(kg, pts)
                if kg % 2 == pair:
                    emit_outproj_unit()

            # drain U banks to SBUF immediately (frees PSUM for the next
            # pair); normalize off the critical path
            ucp = {}
            for h in heads:
                u = sb.tile([65, QC], F32, tag="ucp", bufs=4, name=f"ucp{qc}_{h}")
                nc.vector.tensor_copy(u[:], U[h][0:65, :])
                ucp[h] = u
            z2 = sb.tile([2, QC], F32, tag="z2", bufs=3, name=f"z2_{qc}_{pair}")
            for i, h in enumerate(heads):
                nc.sync.dma_start(z2[i : i + 1, :], ucp[h][64:65, :])
            rz2 = sb.tile([2, QC], F32, tag="rz2", bufs=3, name=f"rz2_{qc}_{pair}")
            nc.vector.reciprocal_approx_fast(rz2[:], z2[:])
            UN[qc, pair] = sb.tile([P, QC], BF16, tag="un", bufs=8, name=f"UN{qc}_{pair}")
            for i, h in enumerate(heads):
                off = (h % 2) * 64
                if i == 0:
                    r0 = rz2[0:1, :]
                else:
                    r0t = sb.tile([1, QC], F32, tag="r0", bufs=3, name=f"r0_{qc}_{h}")
                    nc.sync.dma_start(r0t[:], rz2[1:2, :])
                    r0 = r0t[:]
                rb = sb.tile([64, QC], F32, tag="rb", bufs=4, name=f"rb{qc}_{h}")
                nc.gpsimd.partition_broadcast(rb[:], r0, channels=64)
                if off == 0:
                    nc.vector.tensor_mul(UN[qc, pair][0:64, :], ucp[h][0:64, :], rb[:])
                else:
                    tmp = sb.tile([64, QC], BF16, tag="untmp", bufs=3, name=f"untmp{qc}_{h}")
                    nc.vector.tensor_mul(tmp[:], ucp[h][0:64, :], rb[:])
                    nc.sync.dma_start(UN[qc, pair][64:128, :], tmp[:])

        pending.extend((qc, qi, ec) for qi in range(4) for ec in range(2))

    while pending:
        emit_outproj_unit()


def build_program():
    nc = bacc.Bacc(
        "TRN2", target_bir_lowering=False, debug=False, num_devices=NCORES
    )
    io = {
        "xq": nc.dram_tensor("xq", [D, S], BF16, kind="ExternalInput").ap(),
        "xk": nc.dram_tensor("xk", [D, S], BF16, kind="ExternalInput").ap(),
        "xv": nc.dram_tensor("xv", [D, S], BF16, kind="ExternalInput").ap(),
        "wq": nc.dram_tensor("wq", [D, DG], BF16, kind="ExternalInput").ap(),
        "wk": nc.dram_tensor("wk", [D, DG], BF16, kind="ExternalInput").ap(),
        "wv": nc.dram_tensor("wv", [D, DG], BF16, kind="ExternalInput").ap(),
        "wo": nc.dram_tensor("wo", [DG, D], BF16, kind="ExternalInput").ap(),
        "bq": nc.dram_tensor("bq", [DG], F32, kind="ExternalInput").ap(),
        "bk": nc.dram_tensor("bk", [DG], F32, kind="ExternalInput").ap(),
        "bv": nc.dram_tensor("bv", [DG], BF16, kind="ExternalInput").ap(),
        "ones": nc.dram_tensor("ones", [P], BF16, kind="ExternalInput").ap(),
        "ones4": nc.dram_tensor("ones4", [P, 4], BF16, kind="ExternalInput").ap(),
        "y": nc.dram_tensor("y", [S, D], BF16, kind="ExternalOutput").ap(),
    }
    with tile.TileContext(nc) as tc:
        with ExitStack() as ctx:
            _body(ctx, tc, io)
    nc.compile()
    return nc


_CACHE = {}


def _get_program():
    if "nc" not in _CACHE:
        _CACHE["nc"] = build_program()
    return _CACHE["nc"]


def make_in_maps(inputs):
    q = np.asarray(inputs["query"], np.float32)
    k = np.asarray(inputs["key"], np.float32)
    v = np.asarray(inputs["value"], np.float32)
    W_q = np.asarray(inputs["W_q"], np.float32)
    W_k = np.asarray(inputs["W_k"], np.float32)
    W_v = np.asarray(inputs["W_v"], np.float32)
    W_o = np.asarray(inputs["W_o"], np.float32)
    b_q = np.asarray(inputs["b_q"], np.float32)
    b_k = np.asarray(inputs["b_k"], np.float32)
    b_v = np.asarray(inputs["b_v"], np.float32)

    bf = ml_dtypes.bfloat16
    xT = [
        [np.ascontiguousarray(x[b].T).astype(bf) for b in range(B)]
        for x in (q, k, v)
    ]
    in_maps = []
    for core in range(NCORES):
        b, g = divmod(core, NG)
        sl = slice(g * DG, (g + 1) * DG)
        in_maps.append(
            {
                "xq": xT[0][b],
                "xk": xT[1][b],
                "xv": xT[2][b],
                "wq": np.ascontiguousarray(W_q[sl, :].T).astype(bf),
                "wk": np.ascontiguousarray(W_k[sl, :].T).astype(bf),
                "wv": np.ascontiguousarray(W_v[sl, :].T).astype(bf),
                "wo": np.ascontiguousarray(W_o[:, sl].T).astype(bf),
                "bq": np.ascontiguousarray(b_q[sl]),
                "bk": np.ascontiguousarray(b_k[sl]),
                "bv": np.ascontiguousarray(b_v[sl]).astype(bf),
                "ones": np.ones(P, bf),
                "ones4": np.ones((P, 4), bf),
            }
        )
    return in_maps


def kernel(**inputs):
    from concourse.bass_utils import run_bass_kernel_spmd

    nc = _get_program()
    in_maps = make_in_maps(inputs)
    trace = bool(int(os.environ.get("MHA_TRACE", "0")))
    res = run_bass_kernel_spmd(nc, in_maps, list(range(NCORES)), trace=trace)
    _CACHE["last_results"] = res

    b_o = np.asarray(inputs["b_o"], np.float32)
    out = np.zeros((B, S, D), np.float32)
    for core in range(NCORES):
        b = core // NG
        out[b] += res.results[core]["y"].astype(np.float32)
    out += b_o[None, None, :]
    return out


# revision 29
# speedup vs baseline: 1.1903x; 1.1900x over previous
"""Multi-head attention (B=2, S=2048, D=1024, H=16) on 8 Trainium2 cores.

Sharding: core = 4*b + g  (b = batch 0..1, g = head-group 0..3, 4 heads each).
Each core computes, for its batch b and head-group g (256 of the 1024 dims):
  QT/KT = (x @ W^T)^T  in [d, s] layout   (d on partitions)
  V     = x @ W^T      in [s, d] layout   (s on partitions)
  ST    = scores^T     in [k, q] layout   (k on partitions)  -> exp on ACT
  U     = V^T @ P^T    in [d, q] layout + per-head denominators Z via ones-matmul
  UN    = U / Z        (PE-broadcast reciprocal, DVE multiply)
  Ypart = UN^T @ WoT   in [q, e] layout   (partial over this group's 256 dims)
Host sums the 4 per-group partials per batch and adds b_o.

Schedule (v2): the ACT engine's exp stream (137us total) is the binding
resource, so everything is arranged to start it early and never starve it:
  - projections run Q -> K (shared PSUM ring), attention starts right after K
  - V is projected one sp-block per k-group, interleaved into the first
    attention pair's k-sweep on the PE queue
  - the two heads of a pair issue scores back-to-back at row groups 0/64
    (c=64 matmuls at disjoint tile_position rows execute concurrently)
  - U banks are drained to SBUF immediately at sweep end; the reciprocal/
    broadcast/multiply normalization runs off the critical path
  - output partials are written bf16 (host accumulates in f32)
"""

import os
from contextlib import ExitStack

import ml_dtypes
import numpy as np

import concourse.bass as bass
import concourse.tile as tile
from concourse import bacc, mybir
from concourse.tile import add_dep_helper

B, S, D = 2, 2048, 1024
H, DH = 16, 64
NCORES = 8
NG = 4                  # head-group shards
DG = D // NG            # 256 dims per head-group (4 heads)
P = 128
QC = 512                # q-chunk width
NQC = S // QC           # 4
NKT = S // P            # 16 k-tiles of 128
CD = D // P             # 8 contraction tiles for the projections
F32 = mybir.dt.float32
BF16 = mybir.dt.bfloat16
AF = mybir.ActivationFunctionType
SCALE = 1.0 / float(np.sqrt(D))


def _body(ctx: ExitStack, tc: "tile.TileContext", io: dict):
    nc = tc.nc
    ctx.enter_context(nc.allow_low_precision(reason="bf16 matmul pipeline"))
    sb = ctx.enter_context(tc.tile_pool(name="sb", bufs=1))

    QT, KT = {}, {}
    V = {}

    # --- phase 1: Q then K projections (shared 8-bank PSUM ring) -----------
    with tc.tile_pool(name="ps_proj", bufs=1, space="PSUM") as ps1:
        for nm, xkey, wkey, bkey, xtag, outmap in (
            ("q", "xq", "wq", "bq", "xq", QT),
            ("k", "xk", "wk", "bk", "xk", KT),
        ):
            w = sb.tile([P, CD, DG], BF16, tag=f"w{nm}", bufs=1, name=f"w{nm}")
            nc.sync.dma_start(w[:], io[wkey].rearrange("(c p) d -> p c d", p=P))
            psg = {}
            for d in range(2):
                for sc in range(NQC):
                    psg[d, sc] = ps1.tile(
                        [P, QC], F32, tag="proj", bufs=8, name=f"ps_{nm}{d}{sc}"
                    )
            bias = sb.tile([P, 2], F32, tag=f"b{nm}", bufs=1, name=f"b{nm}")
            xts = []
            for c in range(CD):
                xt = sb.tile([P, S], BF16, tag=xtag, bufs=8, name=f"x{nm}{c}")
                nc.sync.dma_start(xt[:], io[xkey][c * P : (c + 1) * P, :])
                xts.append(xt)
                if c == 0:
                    nc.sync.dma_start(
                        bias[:], io[bkey].rearrange("(t p) -> p t", p=P)
                    )
                for d in range(2):
                    for sc in range(NQC):
                        nc.tensor.matmul(
                            psg[d, sc][:],
                            (w[:, c, d * P : (d + 1) * P]),
                            (xt[:, sc * QC : (sc + 1) * QC]),
                            start=(c == 0),
                            stop=(c == CD - 1),
                        )
            if nm == "k":
                # issue V-phase + attention-phase DMAs while K computes
                wv = sb.tile([P, CD, DG], BF16, tag="wv", bufs=1, name="wv")
                nc.sync.dma_start(wv[:], io["wv"].rearrange("(c p) d -> p c d", p=P))
                xvs = []
                for c in range(CD):
                    xvt = sb.tile([P, S], BF16, tag="xv", bufs=8, name=f"xv{c}")
                    nc.sync.dma_start(xvt[:], io["xv"][c * P : (c + 1) * P, :])
                    xvs.append(xvt)
                ones_col = sb.tile([1, P], BF16, tag="ones_col", bufs=1, name="ones_col")
                nc.sync.dma_start(ones_col[:], io["ones"][None, :])
                ones4 = sb.tile([P, 4], BF16, tag="ones4", bufs=1, name="ones4")
                nc.sync.dma_start(ones4[:], io["ones4"][:])
                bv_row = sb.tile([1, DG], BF16, tag="bv", bufs=1, name="bv_row")
                nc.sync.dma_start(bv_row[:], io["bv"][None, :])
                woT = []
                for pr in range(2):
                    t = sb.tile([P, D], BF16, tag="wo", bufs=2, name=f"woT{pr}")
                    nc.sync.dma_start(t[:], io["wo"][pr * P : (pr + 1) * P, :])
                    woT.append(t)
            # d-outer bias order: pair 0's scores only need the d=0 tiles
            for d in range(2):
                for sc in range(NQC):
                    t = sb.tile([P, QC], BF16, tag=f"{nm}t", bufs=8, name=f"{nm}T{d}{sc}")
                    nc.vector.tensor_scalar_add(t[:], psg[d, sc][:], bias[:, d : d + 1])
                    outmap[d, sc] = t

    # --- phase 2: attention; V projection interleaved into (qc0, pair0) ----
    # PSUM budget: bank1(V-proj psv / out-proj yps) 2 + st 2x2 + u 2 = 8.
    ps2 = ctx.enter_context(tc.tile_pool(name="ps_attn", bufs=1, space="PSUM"))
    UN = {}
    YSB = {}
    Y0 = {}
    pending = []

    def v_block(sp):
        # V [2048, 256]: two 128-row s-tiles per PSUM bank; bias folded via
        # ones-matmul seeds. V_aug tiles [128, 4, 65]: per head 64 V columns
        # + a ones column that accumulates the softmax denominator.
        psv = ps2.tile([P, 2, DG], F32, tag="bank1", bufs=2, name=f"psv{sp}")
        seed = None
        for j in range(2):
            mm = nc.tensor.matmul(
                psv[:, j, :],
                (ones_col[:, 0:P]),
                (bv_row[:]),
                start=(j == 0),
                stop=False,
            )
            if j == 0:
                seed = mm
            else:
                add_dep_helper(mm.ins, seed.ins, reason="psum group order")
        last_j0 = None
        for c in range(CD):
            for j in range(2):
                st_i = sp * 2 + j
                mm = nc.tensor.matmul(
                    psv[:, j, :],
                    (xvs[c][:, st_i * P : (st_i + 1) * P]),
                    (wv[:, c, :]),
                    start=False,
                    stop=(c == CD - 1 and j == 1),
                )
                if j == 0:
                    last_j0 = mm
                elif c == CD - 1:
                    add_dep_helper(mm.ins, last_j0.ins, reason="psv stop order")
        for j in range(2):
            vt = sb.tile([P, 4, DH + 1], BF16, tag="v", bufs=16, name=f"V{sp}_{j}")
            nc.vector.tensor_copy(
                vt[:, :, 0:DH],
                psv[:, j, :].rearrange("p (g d) -> p g d", g=4),
            )
            nc.vector.tensor_copy(vt[:, :, DH : DH + 1], ones4[:, :, None])
            V[sp * 2 + j] = vt

    def emit_outproj_unit():
        if not pending:
            return
        qcp, qi, ec = pending.pop(0)
        qt = qcp * 4 + qi
        if ec == 0:
            YSB[qt] = sb.tile([P, D], BF16, tag="y", bufs=4, name=f"Y{qt}")
        ysb = YSB[qt]
        yps = ps2.tile([P, QC], F32, tag="bank1", bufs=2, name=f"yp{qt}_{ec}")
        for pr in range(2):
            nc.tensor.matmul(
                yps[:],
                (UN[qcp, pr][:, qi * P : (qi + 1) * P]),
                (woT[pr][:, ec * QC : (ec + 1) * QC]),
                start=(pr == 0),
                stop=(pr == 1),
            )
        nc.vector.tensor_copy(ysb[:, ec * QC : (ec + 1) * QC], yps[:])
        if ec == 1:
            nc.sync.dma_start(io["y"][qt * P : (qt + 1) * P, :], ysb[:])

    for qc in range(NQC):
        for pair in range(2):
            heads = (2 * pair, 2 * pair + 1)
            U = {
                h: ps2.tile([P, QC], F32, tag="u", bufs=2, name=f"U{qc}_{h}")
                for h in heads
            }
            def emit_pv(kg, pts):
                for h in heads:
                    for kk in range(2):
                        k_tile = kg * 2 + kk
                        nc.tensor.matmul(
                            U[h][0:65, :],
                            (V[k_tile][:, h, :]),
                            (pts[h][:, kk, :]),
                            start=(kg == 0 and kk == 0),
                            stop=(kg == 7 and kk == 1),
                        )

            pts_all = {}
            for kg in range(8):
                st = {
                    h: ps2.tile([P, 2, QC], F32, tag="st", bufs=2, name=f"st{qc}_{kg}_{h}")
                    for h in heads
                }
                # heads alternate row groups 0-63 / 64-127 -> the PE runs the
                # two c=64 scores matmuls of a kk concurrently
                s_last = None
                for kk in range(2):
                    k_tile = kg * 2 + kk
                    sc, off = divmod(k_tile, 4)
                    for h in heads:
                        pr, lo = h // 2, (h % 2) * 64
                        s_last = nc.tensor.matmul(
                            st[h][:, kk, :],
                            (KT[pr, sc][lo : lo + 64, off * P : (off + 1) * P]),
                            (QT[pr, qc][lo : lo + 64, :]),
                            start=True,
                            stop=True,
                            tile_position=(lo, 0),
                        )
                pts = {}
                for h in heads:
                    pt = sb.tile([P, 2, QC], BF16, tag="pt", bufs=8, name=f"pt{qc}_{kg}_{h}")
                    nc.scalar.activation(pt[:], st[h][:], AF.Exp, scale=SCALE)
                    pts[h] = pt
                pts_all[kg] = pts
                if qc == 0 and pair == 0:
                    v_block(kg)
                emit_pv(kg, pts)
                if kg % 2 == pair:
                    emit_outproj_unit()

            # drain U banks to SBUF immediately (frees PSUM for the next
            # pair); normalize off the critical path
            ucp = {}
            for h in heads:
                u = sb.tile([65, QC], F32, tag="ucp", bufs=4, name=f"ucp{qc}_{h}")
                nc.vector.tensor_copy(u[:], U[h][0:65, :])
                ucp[h] = u
            z2 = sb.tile([2, QC], F32, tag="z2", bufs=3, name=f"z2_{qc}_{pair}")
            for i, h in enumerate(heads):
                nc.sync.dma_start(z2[i : i + 1, :], ucp[h][64:65, :])
            rz2 = sb.tile([2, QC], F32, tag="rz2", bufs=3, name=f"rz2_{qc}_{pair}")
            nc.vector.reciprocal_approx_fast(rz2[:], z2[:])
            UN[qc, pair] = sb.tile([P, QC], BF16, tag="un", bufs=8, name=f"UN{qc}_{pair}")
            for i, h in enumerate(heads):
                off = (h % 2) * 64
                if i == 0:
                    r0 = rz2[0:1, :]
                else:
                    r0t = sb.tile([1, QC], F32, tag="r0", bufs=3, name=f"r0_{qc}_{h}")
                    nc.sync.dma_start(r0t[:], rz2[1:2, :])
                    r0 = r0t[:]
                rb = sb.tile([64, QC], F32, tag="rb", bufs=4, name=f"rb{qc}_{h}")
                nc.gpsimd.partition_broadcast(rb[:], r0, channels=64)
                if off == 0:
                    nc.vector.tensor_mul(UN[qc, pair][0:64, :], ucp[h][0:64, :], rb[:])
                else:
                    tmp = sb.tile([64, QC], BF16, tag="untmp", bufs=3, name=f"untmp{qc}_{h}")
                    nc.vector.tensor_mul(tmp[:], ucp[h][0:64, :], rb[:])
                    nc.sync.dma_start(UN[qc, pair][64:128, :], tmp[:])

        pending.extend((qc, qi, ec) for qi in range(4) for ec in range(2))

    while pending:
        emit_outproj_unit()


def build_program():
    nc = bacc.Bacc(
        "TRN2", target_bir_lowering=False, debug=False, num_devices=NCORES
    )
    io = {
        "xq": nc.dram_tensor("xq", [D, S], BF16, kind="ExternalInput").ap(),
        "xk": nc.dram_tensor("xk", [D, S], BF16, kind="ExternalInput").ap(),
        "xv": nc.dram_tensor("xv", [D, S], BF16, kind="ExternalInput").ap(),
        "wq": nc.dram_tensor("wq", [D, DG], BF16, kind="ExternalInput").ap(),
        "wk": nc.dram_tensor("wk", [D, DG], BF16, kind="ExternalInput").ap(),
        "wv": nc.dram_tensor("wv", [D, DG], BF16, kind="ExternalInput").ap(),
        "wo": nc.dram_tensor("wo", [DG, D], BF16, kind="ExternalInput").ap(),
        "bq": nc.dram_tensor("bq", [DG], F32, kind="ExternalInput").ap(),
        "bk": nc.dram_tensor("bk", [DG], F32, kind="ExternalInput").ap(),
        "bv": nc.dram_tensor("bv", [DG], BF16, kind="ExternalInput").ap(),
        "ones": nc.dram_tensor("ones", [P], BF16, kind="ExternalInput").ap(),
        "ones4": nc.dram_tensor("ones4", [P, 4], BF16, kind="ExternalInput").ap(),
        "y": nc.dram_tensor("y", [S, D], BF16, kind="ExternalOutput").ap(),
    }
    with tile.TileContext(nc) as tc:
        with ExitStack() as ctx:
            _body(ctx, tc, io)
    nc.compile()
    return nc


_CACHE = {}


def _get_program():
    if "nc" not in _CACHE:
        _CACHE["nc"] = build_program()
    return _CACHE["nc"]


def make_in_maps(inputs):
    q = np.asarray(inputs["query"], np.float32)
    k = np.asarray(inputs["key"], np.float32)
    v = np.asarray(inputs["value"], np.float32)
    W_q = np.asarray(inputs["W_q"], np.float32)
    W_k = np.asarray(inputs["W_k"], np.float32)
    W_v = np.asarray(inputs["W_v"], np.float32)
    W_o = np.asarray(inputs["W_o"], np.float32)
    b_q = np.asarray(inputs["b_q"], np.float32)
    b_k = np.asarray(inputs["b_k"], np.float32)
    b_v = np.asarray(inputs["b_v"], np.float32)

    bf = ml_dtypes.bfloat16
    xT = [
        [np.ascontiguousarray(x[b].T).astype(bf) for b in range(B)]
        for x in (q, k, v)
    ]
    in_maps = []
    for core in range(NCORES):
        b, g = divmod(core, NG)
        sl = slice(g * DG, (g + 1) * DG)
        in_maps.append(
            {
                "xq": xT[0][b],
                "xk": xT[1][b],
                "xv": xT[2][b],
                "wq": np.ascontiguousarray(W_q[sl, :].T).astype(bf),
                "wk": np.ascontiguousarray(W_k[sl, :].T).astype(bf),
                "wv": np.ascontiguousarray(W_v[sl, :].T).astype(bf),
                "wo": np.ascontiguousarray(W_o[:, sl].T).astype(bf),
                "bq": np.ascontiguousarray(b_q[sl]),
                "bk": np.ascontiguousarray(b_k[sl]),
                "bv": np.ascontiguousarray(b_v[sl]).astype(bf),
                "ones": np.ones(P, bf),
                "ones4": np.ones((P, 4), bf),
            }
        )
    return in_maps


def kernel(**inputs):
    from concourse.bass_utils import run_bass_kernel_spmd

    nc = _get_program()
    in_maps = make_in_maps(inputs)
    trace = bool(int(os.environ.get("MHA_TRACE", "0")))
    res = run_bass_kernel_spmd(nc, in_maps, list(range(NCORES)), trace=trace)
    _CACHE["last_results"] = res

    b_o = np.asarray(inputs["b_o"], np.float32)
    out = np.zeros((B, S, D), np.float32)
    for core in range(NCORES):
        b = core // NG
        out[b] += res.results[core]["y"].astype(np.float32)
    out += b_o[None, None, :]
    return out
